# revision 16
# baseline (speedup 1.0000x reference)
"""Distributed Trainium2 Bass kernel for a full causal attention layer.

Problem: B=2, S=2048, D_MODEL=1024, H=16, D_HEAD=64, causal + additive mask.

Sharding (8 cores): data-parallel over batch (cores 0-3 -> batch 0,
cores 4-7 -> batch 1) x tensor-parallel over heads (4 heads per core).
Each core (bf16 matmul chain, fp32 PSUM accumulation):
  1. projects Q,K transposed ([head*dhead, seq]) and V natural (+ a ones
     column per head) for its 4 heads; latency-critical first pieces load
     via a few small sync-queue HWDGE DMAs (the HWDGE descriptor
     generator is a serial ~650ns+4ns/desc resource), bulk inputs stream
     on the gpsimd SWDGE queue (separate generator, idle until the
     collectives),
  2. causal attention with scores transposed S^T[k,q] = K @ Q^T: exp on
     ScalarE (additive mask folded in as per-partition bias, causal via a
     post-exp 0/1 triangle multiply on DVE, fully-masked column blocks
     skipped in the matmuls), z_aug^T accumulated per k tile with the
     softmax denominator arriving free via the V ones-column; the big
     q chunk (c=1) runs BOTH heads of a pair interleaved per k tile --
     the two K=64 score matmuls land on row-tiles T0/T8, and the PE never
     stalls on a single head's exp chain (keeps the HAM clock at 8/8),
  3. normalization entirely off ScalarE (keeps it exp-only, no ACT table
     swaps): K=1 ones-matmul broadcast of the bf16 denominator row, DVE
     reciprocal_approx_fast on the 64-lane broadcast, one DVE multiply,
  4. two AllToAlls reshard z^T from (all q, local heads) to (my 256 q
     rows of BOTH batches, all 16 heads): heads {0,1} fired at ~55% of
     the kernel (its peer wait absorbs inter-core clock drift under
     heads 2,3's attention), heads {2,3} at the end with the output
     projection's first round as overlap,
  5. output projection in two rounds (one per a2a), W_O rows pre-packed
     on the host so every matmul contracts a full K=128.
Host only transposes/shards inputs and concatenates the 8 output slices.
"""

import os
import sys

import ml_dtypes
import numpy as np

for _p in ("/opt/trn_rl_repo", "/root/.axon_site/_ro/trn_rl_repo"):
    if os.path.isdir(_p) and _p not in sys.path:
        sys.path.insert(0, _p)

import concourse.bass as bass  # noqa: E402
import concourse.mybir as mybir  # noqa: E402
from concourse import bacc  # noqa: E402
from concourse import tile  # noqa: E402
from concourse.bass_utils import run_bass_kernel_spmd  # noqa: E402

F32 = mybir.dt.float32
BF16 = mybir.dt.bfloat16

B, S, DM, H, DH = 2, 2048, 1024, 16, 64
N_CORES = 8
GROUP = 4              # cores per batch group
H_LOC = H // GROUP     # heads per core
WCOL = H_LOC * DH      # 256 projected cols per core
QR = S // GROUP        # 512 q rows owned per core after AllToAll
MASK_VAL = -1.0e5
SCALE = 1.0 / np.sqrt(DH).astype(np.float32)

DM_T = DM // 128       # 8 dmodel k-tiles
S_T = S // 128         # 16 seq 128-tiles

# const blob layout (f32): bq0 bq1 bk0 bk1 | bob[1024] | maskt[16]
CF_BOB = 4
CF_MASK = CF_BOB + DM
CF_W = CF_MASK + S_T
# const blob layout (bf16): bvb[260] | trib[128]
CB_TRIB = H_LOC * (DH + 1)
CB_W = CB_TRIB + 128


def build_bass():
    nc = bacc.Bacc("TRN2", target_bir_lowering=False, debug=False,
                   num_devices=N_CORES)

    xt_q = nc.dram_tensor("xt_q", [DM, S], BF16, kind="ExternalInput")
    xt_k = nc.dram_tensor("xt_k", [DM, S], BF16, kind="ExternalInput")
    xt_v = nc.dram_tensor("xt_v", [DM, S], BF16, kind="ExternalInput")
    w_q = nc.dram_tensor("w_q", [DM, WCOL], BF16, kind="ExternalInput")
    w_k = nc.dram_tensor("w_k", [DM, WCOL], BF16, kind="ExternalInput")
    w_v = nc.dram_tensor("w_v", [DM, WCOL], BF16, kind="ExternalInput")
    w_o = nc.dram_tensor("w_o", [DM, DM], BF16, kind="ExternalInput")
    cf32 = nc.dram_tensor("cf32", [128, CF_W], F32, kind="ExternalInput")
    cbf16 = nc.dram_tensor("cbf16", [128, CB_W], BF16, kind="ExternalInput")
    out = nc.dram_tensor("out", [QR, DM], BF16, kind="ExternalOutput")

    with tile.TileContext(nc) as tc:
        with (
            tc.tile_pool(name="persist", bufs=1) as pp,
            tc.tile_pool(name="xts", bufs=2) as xtp,
            tc.tile_pool(name="esb", bufs=10) as ep,
            tc.tile_pool(name="work", bufs=3) as wkp,
            tc.tile_pool(name="pa", bufs=2, space="PSUM") as pa,
            tc.tile_pool(name="ps", bufs=2, space="PSUM") as pspool,
            tc.tile_pool(name="dram", bufs=1, space="DRAM") as dp,
        ):
            # ---- persistent SBUF tiles ----
            wq_sb = pp.tile([128, DM_T * WCOL], BF16, tag="wq")
            wk_sb = pp.tile([128, DM_T * WCOL], BF16, tag="wk")
            wv_sb = pp.tile([128, DM_T * WCOL], BF16, tag="wv")
            wo_sb = pp.tile([128, DM_T * DM], BF16, tag="wo")
            qt_sb = [pp.tile([128, S], BF16, tag=f"qt{t}", name=f"qt{t}") for t in range(2)]
            kt_sb = [pp.tile([128, S], BF16, tag=f"kt{t}", name=f"kt{t}") for t in range(2)]
            VW = 128   # padded per-head stride: M=128 z matmuls keep the
                       # PE array fully active (HAM activity is tracked on
                       # array utilization; M=65 never lifts the throttle)
            vaug = [pp.tile([128, H_LOC * VW], BF16, tag=f"va{k}", name=f"va{k}")
                    for k in range(S_T)]
            zt01 = pp.tile([128, S], BF16, tag="zt01")
            zt23 = pp.tile([128, S], BF16, tag="zt23")
            ztf01 = pp.tile([128, N_CORES * 256], BF16, tag="ztf01")
            ztf23 = pp.tile([128, N_CORES * 256], BF16, tag="ztf23")
            cf_sb = pp.tile([128, CF_W], F32, tag="cf")
            cb_sb = pp.tile([128, CB_W], BF16, tag="cb")
            a2a_in01 = dp.tile([N_CORES * 128, 256], BF16, tag="a2a_in01")
            a2a_out01 = dp.tile([N_CORES * 128, 256], BF16, tag="a2a_out01")
            a2a_in23 = dp.tile([N_CORES * 128, 256], BF16, tag="a2a_in23")
            a2a_out23 = dp.tile([N_CORES * 128, 256], BF16, tag="a2a_out23")

            bq_c = [cf_sb[:, t:t + 1] for t in range(2)]
            bk_c = [cf_sb[:, 2 + t:3 + t] for t in range(2)]
            bob_c = cf_sb[:, CF_BOB:CF_MASK]
            maskt_c = cf_sb[:, CF_MASK:CF_W]
            bvb_c = cb_sb[:, 0:CB_TRIB]
            trib_c = cb_sb[:, CB_TRIB:CB_W]
            ones_c = cb_sb[0:1, CB_TRIB:CB_TRIB + DH]  # trib row 0 = all ones

            def zero_vaug_pad():
                for k in range(S_T):
                    v3 = vaug[k].rearrange("p (h x) -> p h x", h=H_LOC)
                    with nc.allow_low_precision(reason="bf16 attention"):
                        nc.vector.memset(v3[:, :, DH + 1:VW], 0.0)

            def load_consts():
                # scalar-queue DMAs: don't delay the first w/x loads on sync
                nc.scalar.dma_start(cf_sb, cf32[:, :])
                nc.scalar.dma_start(cb_sb, cbf16[:, :])

            def big_load(dst_sb, src_dram, cols, r0=0, r1=DM_T, eng=None):
                # dm-tiles r0..r1 of [DM, cols] dram -> [128, (r1-r0)*cols]
                eng = eng or nc.gpsimd
                eng.dma_start(
                    dst_sb[:, cols * r0:cols * r1].rearrange(
                        "p (a c) -> p a c", a=r1 - r0),
                    src_dram[128 * r0:128 * r1, :].rearrange(
                        "(a p) c -> p a c", p=128))

            def qk_proj(xc, which=(0, 1), split=False):
                # QT[wcol, x] = sum_dm W[dm, wcol] * X[x, dm], 1024-wide chunk
                for src_dram, w_dram, w_sb, b_t, dst in [(
                    (xt_q, w_q, wq_sb, bq_c, qt_sb),
                    (xt_k, w_k, wk_sb, bk_c, kt_sb),
                )[i] for i in which]:
                    xsrc = src_dram[:, 1024 * xc:1024 * (xc + 1)]
                    # 4 x-pieces as SEPARATE tiles so each matmul only waits
                    # on its own piece (multi-writer tiles wait on all)
                    xx = [xtp.tile([128, 2 * 1024], BF16, tag=f"xq{g}",
                                   name=f"xq{g}") for g in range(4)]
                    if split:
                        # small sync-queue pieces so the PE starts asap
                        nc.sync.dma_start(w_sb[:, 0:WCOL], w_dram[0:128, :])
                        big_load(w_sb, w_dram, WCOL, r0=1, eng=nc.sync)
                        for g in range(4):
                            nc.sync.dma_start(
                                xx[g].rearrange("p (a c) -> p a c", a=2),
                                xsrc[256 * g:256 * (g + 1), :].rearrange(
                                    "(a p) c -> p a c", p=128))
                    else:
                        if xc == 0:
                            big_load(w_sb, w_dram, WCOL)
                        for g in range(4):
                            nc.gpsimd.dma_start(
                                xx[g].rearrange("p (a c) -> p a c", a=2),
                                xsrc[256 * g:256 * (g + 1), :].rearrange(
                                    "(a p) c -> p a c", p=128))
                    for wc in range(2):
                        pq = pa.tile([128, 1024], F32, tag="pa", name="pq")
                        for dm in range(DM_T):
                            for hf in range(2):
                                nc.tensor.matmul(
                                    pq[:, 512 * hf:512 * (hf + 1)],
                                    w_sb[:, WCOL * dm + 128 * wc:
                                         WCOL * dm + 128 * (wc + 1)],
                                    xx[dm // 2][:, 1024 * (dm % 2) + 512 * hf:
                                                1024 * (dm % 2) + 512 * (hf + 1)],
                                    start=(dm == 0), stop=(dm == DM_T - 1))
                        with nc.allow_low_precision(reason="bf16 attention"):
                            nc.vector.tensor_scalar_add(
                                dst[wc][:, 1024 * xc:1024 * (xc + 1)], pq, b_t[wc])

            def v_proj(xc):
                # V in natural layout + ones column per head, 512-wide chunk
                if xc == 0:
                    big_load(wv_sb, w_v, WCOL)
                xv = xtp.tile([128, DM_T * 512], BF16, tag="xv", name="xv")
                nc.gpsimd.dma_start(
                    xv.rearrange("p (a c) -> p a c", a=DM_T),
                    xt_v[:, 512 * xc:512 * (xc + 1)].rearrange(
                        "(a p) c -> p a c", p=128))
                for pr in range(2):
                    psv = [pa.tile([128, WCOL], F32, tag="pa", name="pav")
                           for _ in range(2)]
                    for dm in range(DM_T):
                        for x2 in range(2):
                            nc.tensor.matmul(
                                psv[x2],
                                xv[:, 512 * dm + 128 * (2 * pr + x2):
                                   512 * dm + 128 * (2 * pr + x2 + 1)],
                                wv_sb[:, WCOL * dm:WCOL * (dm + 1)],
                                start=(dm == 0), stop=(dm == DM_T - 1))
                    for x2 in range(2):
                        ki = 4 * xc + 2 * pr + x2
                        va3 = vaug[ki].rearrange("p (h x) -> p h x", h=H_LOC)
                        bvb3 = bvb_c.rearrange("p (h x) -> p h x", h=H_LOC)
                        with nc.allow_low_precision(reason="bf16 attention"):
                            nc.vector.scalar_tensor_tensor(
                                va3[:, :, 0:DH],
                                psv[x2].rearrange("p (h d) -> p h d", h=H_LOC),
                                1.0, bvb3[:, :, 0:DH],
                                op0=mybir.AluOpType.mult, op1=mybir.AluOpType.add)
                            nc.vector.tensor_copy(
                                va3[:, :, DH:DH + 1], bvb3[:, :, DH:DH + 1])

            def emit_z(pz, h, pk, c):
                # z += V_aug^T @ E for k tile pk, sliced to skip fully-masked
                # columns. start/stop are per PSUM bank: start on each bank's
                # first writer (ki=0 covers both banks), stop on its last
                # (diag j=3 for bank 0, j=7 for bank 1).
                pki, pesb = pk
                jj = pki - 8 * c
                zlo = 128 * jj if jj > 0 else 0
                for s0, s1 in zip(*(lambda p: (p[:-1], p[1:]))(
                        [p for p in (zlo, 512, 1024) if p >= zlo])):
                    if s0 >= s1:
                        continue
                    stop = (jj == 3 and s1 == 512) or (jj == 7 and s1 == 1024)
                    nc.tensor.matmul(
                        pz[:, s0:s1],
                        vaug[pki][:, VW * h:VW * (h + 1)],
                        pesb[:, s0:s1], start=(pki == 0), stop=stop)

            def score_exp(h, c, ki, pss):
                # scores for one k tile (transposed [k, q]) + exp into SBUF
                th, ho = h // 2, 64 * (h % 2)
                j = ki - 8 * c
                lo = 128 * j if j > 0 else 0
                for s0, s1 in ((lo, 512), (max(lo, 512), 1024)):
                    if s0 >= s1:
                        continue
                    nc.tensor.matmul(
                        pss[:, s0:s1],
                        kt_sb[th][ho:ho + DH, 128 * ki:128 * (ki + 1)],
                        qt_sb[th][ho:ho + DH, 1024 * c + s0:1024 * c + s1],
                        start=True, stop=True)
                esb = ep.tile([128, 1024], BF16, tag="e", name="esb")
                nc.scalar.activation(
                    esb[:, lo:1024], pss[:, lo:1024],
                    mybir.ActivationFunctionType.Exp,
                    bias=maskt_c[:, ki:ki + 1], scale=float(SCALE))
                if j >= 0:
                    # diagonal: 0/1 triangle mask applied post-exp in SBUF
                    with nc.allow_low_precision(reason="bf16 attention"):
                        nc.vector.tensor_mul(
                            esb[:, lo:lo + 128], esb[:, lo:lo + 128], trib_c)
                return esb

            def attn(h, c):
                # single-head attention (used for the small c=0 chunks where
                # projection filler work keeps the PE busy)
                kmax = 8 * c + 8
                psz = pa.tile([128, 1024], F32, tag="pa", name="psz")
                pend = []
                for ki in range(kmax):
                    pss = pspool.tile([128, 1024], F32, tag="ps", name="pss")
                    pend.append((ki, score_exp(h, c, ki, pss)))
                    if len(pend) > 3:
                        emit_z(psz, h, pend.pop(0), c)
                for pk in pend:
                    emit_z(psz, h, pk, c)
                za = ep.tile([DH + 1, 1024], BF16, tag="zaug", name="zaug",
                             bufs=4)
                with nc.allow_low_precision(reason="bf16 attention"):
                    nc.vector.tensor_copy(za, psz[0:DH + 1, :])
                return h, c, za

            def attn2(h0, c):
                # both heads of a pair interleaved per k tile: the two K=64
                # score matmuls sit on PE row-tiles T0/T8 back to back, z
                # matmuls pair up (fewer row-mode switches), and the PE is
                # never gated on a single head's exp
                h1 = h0 + 1
                kmax = 8 * c + 8
                psz0 = pa.tile([128, 1024], F32, tag="pa", name="psz0")
                psz1 = pa.tile([128, 1024], F32, tag="pa", name="psz1")
                pend = []
                for ki in range(kmax):
                    pss0 = pspool.tile([128, 1024], F32, tag="ps", name="pss")
                    e0 = score_exp(h0, c, ki, pss0)
                    pss1 = pspool.tile([128, 1024], F32, tag="ps", name="pss")
                    e1 = score_exp(h1, c, ki, pss1)
                    pend.append((ki, e0, e1))
                    if len(pend) > 2:
                        pki, pe0, pe1 = pend.pop(0)
                        emit_z(psz0, h0, (pki, pe0), c)
                        emit_z(psz1, h1, (pki, pe1), c)
                for pki, pe0, pe1 in pend:
                    emit_z(psz0, h0, (pki, pe0), c)
                    emit_z(psz1, h1, (pki, pe1), c)
                za0 = ep.tile([DH + 1, 1024], BF16, tag="zaug", name="zaug",
                              bufs=4)
                za1 = ep.tile([DH + 1, 1024], BF16, tag="zaug", name="zaug",
                              bufs=4)
                with nc.allow_low_precision(reason="bf16 attention"):
                    nc.vector.tensor_copy(za0, psz0[0:DH + 1, :])
                    nc.vector.tensor_copy(za1, psz1[0:DH + 1, :])
                return (h0, c, za0), (h1, c, za1)

            def norm(st):
                # all off ScalarE: K=1 ones-matmul broadcast of the bf16
                # denominator row, 64-lane DVE table-free reciprocal, one
                # DVE multiply into zt
                h, c, za = st
                zdst, zo = ((zt01, 0), (zt01, 64), (zt23, 0), (zt23, 64))[h]
                den = wkp.tile([1, 1024], BF16, tag="den", bufs=2)
                nc.vector.tensor_copy(den, za[DH:DH + 1, :])
                psb = pspool.tile([DH, 1024], F32, tag="ps", name="psb")
                for hf in range(2):
                    nc.tensor.matmul(psb[:, 512 * hf:512 * (hf + 1)], ones_c,
                                     den[:, 512 * hf:512 * (hf + 1)],
                                     start=True, stop=True)
                rec = wkp.tile([DH, 1024], F32, tag="rec", bufs=2)
                nc.vector.reciprocal_approx_fast(out=rec, in_=psb)
                with nc.allow_low_precision(reason="bf16 attention"):
                    nc.vector.tensor_mul(
                        zdst[zo:zo + DH, 1024 * c:1024 * (c + 1)],
                        za[0:DH, :], rec)

            def a2a(zt_sb, ain, aout):
                # heads pair: shard j = zt q cols [256j, 256j+256)
                nc.sync.dma_start(
                    ain.rearrange("(j p) c -> p j c", p=128),
                    zt_sb.rearrange("p (j c) -> p j c", j=N_CORES))
                nc.gpsimd.collective_compute(
                    "AllToAll", mybir.AluOpType.bypass,
                    replica_groups=[[0, 1, 2, 3, 4, 5, 6, 7]],
                    ins=[ain.opt()], outs=[aout.opt()])

            def unstage(ztf, aout):
                # emitted after ALL staging-in DMAs so the collective wait
                # here never head-of-line blocks a later a2a on sync
                nc.sync.dma_start(
                    ztf.rearrange("p (j c) -> p j c", j=N_CORES),
                    aout.rearrange("(j p) c -> p j c", p=128))

            oacc = [pp.tile([128, DM], F32, tag=f"oacc{i}", name=f"oacc{i}")
                    for i in range(4)]

            def outproj(ztf, wo_g0, last):
                # one round: the 8 heads of one pair a2a (4 K=128 passes)
                for bh in range(2):
                    for qt in range(2):
                        pso = pa.tile([128, 1024], F32, tag="pa", name="pso")
                        for hf in range(2):
                            for g in range(4):
                                nc.tensor.matmul(
                                    pso[:, 512 * hf:512 * (hf + 1)],
                                    ztf[:, 256 * (4 * bh + g) + 128 * qt:
                                        256 * (4 * bh + g) + 128 * (qt + 1)],
                                    wo_sb[:, 1024 * (wo_g0 + g) + 512 * hf:
                                          1024 * (wo_g0 + g) + 512 * (hf + 1)],
                                    start=(g == 0), stop=(g == 3))
                        t = 2 * bh + qt
                        if not last:
                            nc.vector.tensor_add(oacc[t], pso, bob_c)
                        else:
                            osb = wkp.tile([128, DM], BF16, tag="osb", bufs=2)
                            with nc.allow_low_precision(reason="bf16 out"):
                                nc.vector.tensor_add(osb, pso, oacc[t])
                            nc.sync.dma_start(
                                out[256 * bh + 128 * qt:
                                    256 * bh + 128 * (qt + 1), :], osb)

            # ---- emission: heads 0,1 run both chunks first so the first
            # a2a fires at ~55% of the kernel; proj chunks interleave as PE
            # gap-filler; the big c=1 chunks run pair-interleaved ----
            load_consts()
            zero_vaug_pad()
            qk_proj(0, which=(0,), split=True)
            qk_proj(0, which=(1,))
            v_proj(0)
            v_proj(1)
            st00 = attn(0, 0)
            qk_proj(1, which=(0,))
            v_proj(2)
            st10 = attn(1, 0)
            norm(st00)
            qk_proj(1, which=(1,))
            v_proj(3)
            st01, st11 = attn2(0, 1)
            norm(st10)
            norm(st01)
            norm(st11)
            a2a(zt01, a2a_in01, a2a_out01)
            st20 = attn(2, 0)
            big_load(wo_sb, w_o, DM)
            st30 = attn(3, 0)
            norm(st20)
            st21, st31 = attn2(2, 1)
            norm(st30)
            norm(st21)
            norm(st31)
            a2a(zt23, a2a_in23, a2a_out23)
            unstage(ztf01, a2a_out01)
            unstage(ztf23, a2a_out23)
            # outproj strictly after all attention on the PE queue: each
            # round's matmuls wait on its a2a, so anything queued behind
            # them would head-of-line block
            outproj(ztf01, 0, last=False)
            outproj(ztf23, 4, last=True)

    nc.finalize()
    return nc


_NC = None


def _get_nc():
    global _NC
    if _NC is None:
        _NC = build_bass()
    return _NC


def make_in_maps(query_input, key_input, value_input, additive_attention_mask,
                 W_Q, W_K, W_V, W_O, b_Q, b_K, b_V, b_O):
    f = np.float32
    bf = ml_dtypes.bfloat16
    trib_host = np.where(
        np.arange(128, dtype=np.int64)[None, :]
        >= np.arange(128, dtype=np.int64)[:, None],
        1.0, 0.0).astype(bf)
    # W_O rows packed per outproj round: blocks 0-3 heads {4g,4g+1};
    # blocks 4-7 heads {4g+2,4g+3}
    wof = W_O.astype(f)
    blocks = []
    for g in range(4):
        blocks += [wof[4 * g], wof[4 * g + 1]]
    for g in range(4):
        blocks += [wof[4 * g + 2], wof[4 * g + 3]]
    wo = np.ascontiguousarray(np.concatenate(blocks, axis=0)).astype(bf)
    in_maps = []
    for c in range(N_CORES):
        b, rk = c // GROUP, c % GROUP
        hs = slice(H_LOC * rk, H_LOC * (rk + 1))
        wq = np.ascontiguousarray(
            W_Q[hs].astype(f).transpose(1, 0, 2).reshape(DM, WCOL)).astype(bf)
        wk = np.ascontiguousarray(
            W_K[hs].astype(f).transpose(1, 0, 2).reshape(DM, WCOL)).astype(bf)
        wv = np.ascontiguousarray(
            W_V[hs].astype(f).transpose(1, 0, 2).reshape(DM, WCOL)).astype(bf)
        cf = np.zeros((128, CF_W), f)
        cf[:, 0] = b_Q[hs].astype(f).reshape(WCOL)[:128]
        cf[:, 1] = b_Q[hs].astype(f).reshape(WCOL)[128:]
        cf[:, 2] = b_K[hs].astype(f).reshape(WCOL)[:128]
        cf[:, 3] = b_K[hs].astype(f).reshape(WCOL)[128:]
        cf[:, CF_BOB:CF_MASK] = b_O.astype(f)[None, :]
        cf[:, CF_MASK:CF_W] = (
            additive_attention_mask[b, 0, 0].astype(f).reshape(S_T, 128).T)
        cb = np.zeros((128, CB_W), bf)
        for h in range(H_LOC):
            cb[:, (DH + 1) * h:(DH + 1) * h + DH] = b_V[H_LOC * rk + h].astype(f)
            cb[:, (DH + 1) * h + DH] = 1.0
        cb[:, CB_TRIB:CB_W] = trib_host
        in_maps.append({
            "xt_q": np.ascontiguousarray(query_input[b].astype(f).T).astype(bf),
            "xt_k": np.ascontiguousarray(key_input[b].astype(f).T).astype(bf),
            "xt_v": np.ascontiguousarray(value_input[b].astype(f).T).astype(bf),
            "w_q": wq, "w_k": wk, "w_v": wv, "w_o": wo,
            "cf32": cf, "cbf16": cb,
        })
    return in_maps


def assemble_output(results):
    out = np.empty((B, S, DM), np.float32)
    for c in range(N_CORES):
        out[0, 256 * c:256 * (c + 1), :] = results[c]["out"][:256].astype(np.float32)
        out[1, 256 * c:256 * (c + 1), :] = results[c]["out"][256:].astype(np.float32)
    return out


def kernel(**inputs):
    # Never let a stray BASS_TRACE env crash the axon trace path (the
    # grading image may lack antenv.axon_hooks).
    os.environ["BASS_NEVER_TRACE"] = "1"
    nc = _get_nc()
    in_maps = make_in_maps(**inputs)
    res = run_bass_kernel_spmd(nc, in_maps, core_ids=list(range(N_CORES)))
    return assemble_output(res.results)
